# revision 33
# baseline (speedup 1.0000x reference)
"""DPQ (gumbel-softmax product-quantizer autoencoder) forward pass on 8
Trainium2 NeuronCores, data-parallel over the batch dimension.

Math (per row n, subspace m of 8, codebook of K=512 64-dim codes):
  h = x @ W_enc + b_enc                     [N, M*DSUB]
  score = (-|h|^2 + 2 h.c - |c|^2) / T_m    squared-distance scores
  codes = softmax(score + gumbel)           gumbel-softmax, TAU=1
  y = (codes @ C).flatten() @ W_dec + b_dec

v2 implementation notes (HW-profiled against the v1 kernel):
  * Everything runs transposed (n on the free dim); host pre-transposes all
    operands and untransposes the output, so no PE transposes anywhere.
  * The -|h|^2 term is constant over k so softmax cancels it; never computed.
  * The -invT*|c|^2 score bias is folded into the gumbel tensor ON THE HOST
    (free), so ACT exp needs no per-(m,kc) bias and can process both
    subspaces of a pair in one [128,1024] op spanning a 2-bank PSUM tile.
  * All matmul operands are 16-bit (fp16 for x/W/h/scores, bf16 where the
    fp32 exponent range is needed: exp outputs and the codebook in the recon
    matmul).  16-bit weights get a separate LDWEIGHTS that the PE's reorder
    window pulls ahead of in-flight matmuls, so weight loads cost ~0
    (f32r/f32 weights self-load serially, +60ns per matmul on HW).
  * Score matmuls run as 2x2 quad-tiled 64x64 PE tiles: m0 uses PE rows
    0-63, m1 rows 64-127, k-halves split over PE column groups.  All four
    64-contract streams run CONCURRENTLY (profiled ~234ns per quad vs
    ~430ns for the v1 row-paired version).
  * Gumbel noise (with the folded bias) rides identity matmuls into the
    PSUM accumulation for kc<3 and a DVE add for kc==3 (engine balance).
  * Softmax denominator s rides the recon matmul as a ones column (row 64
    of the 65-row up tiles).  1/s is computed by reciprocal_approx_fast
    (~600ns vs 3.4us for exact DVE reciprocal; 18 good bits), reading the
    PSUM row directly; the recon scaling multiply then reads the PSUM up
    tile and a 0-stride partition-broadcast AP of 1/s, fusing the psum
    drain, normalization and fp16 downcast into one DVE op per subspace.
  * Output is written fp16 (the v1 bf16 output cast dominated its error).
    End-to-end rel_absmax vs the fp32 reference: ~5e-3 (validated in numpy
    simulation and on HW), inside the 2e-2 gate.
"""

import sys
sys.path.insert(0, '/opt/trn_rl_repo')

import numpy as np
import ml_dtypes

N, D, M, K, DSUB = 32768, 512, 8, 512, 64
NCORES = 8
NLOC = N // NCORES          # rows per core
BLK = 512                   # rows per block
JC = D // 128               # 4 column chunks of 128
KC = K // 128               # 4 code chunks of 128
MC = M // 2                 # 4 subspace pairs

_CACHE = {}


def build(nblk: int):
    import concourse.bacc as bacc_mod
    import concourse.tile as tile
    import concourse.mybir as mybir
    from concourse.bass import ts
    from concourse.masks import make_identity
    from contextlib import ExitStack

    F32 = mybir.dt.float32
    F16 = mybir.dt.float16
    BF16 = mybir.dt.bfloat16
    AF = mybir.ActivationFunctionType

    nc = bacc_mod.Bacc(trn_type="TRN2", target_bir_lowering=False, debug=False)

    # Host-prepared layouts (see make_in_maps):
    #   XT[p, b, dc, j]      = fp16(x[b*512+j, dc*128+p])
    #   WENC[p, dc, j]       = fp16(W_enc[dc*128+p, j])
    #   AUX[p, 0:4]=b_enc, AUX[p, 4:8]=b_dec (per 128-chunk)
    #   CT2[p, mc, kc, k]    = fp16(2*invT_m*C[m, kc*128+k, d]), m/d from p
    #   CONES[p, m, kc, 0:64]= bf16(C[m, kc*128+p, :]), [...,64] = 1
    #   GT[m, p, b, kc, j]   = fp16(g[b*512+j, m, kc*128+p] - invT_m|c|^2)
    #   WDEC[p, mc, j]       = fp16(W_dec[mc*128+p, j])
    #   YT[p, b, jc, j]      = y[b*512+j, jc*128+p]            (output)
    XT = nc.dram_tensor("xt", [128, nblk, JC, BLK], F16,
                        kind="ExternalInput").ap()
    WENC = nc.dram_tensor("w_enc", [128, JC, D], F16,
                          kind="ExternalInput").ap()
    CT2 = nc.dram_tensor("ct2", [128, MC, KC, 128], F16,
                         kind="ExternalInput").ap()
    CONES = nc.dram_tensor("cones", [128, M, KC, 128], BF16,
                           kind="ExternalInput").ap()
    GT = nc.dram_tensor("gumbel", [MC, 128, nblk, KC, 2, BLK], F16,
                        kind="ExternalInput").ap()
    WDEC = nc.dram_tensor("w_dec", [128, JC, D], F16,
                          kind="ExternalInput").ap()
    YT = nc.dram_tensor("yt", [128, nblk, JC, BLK], F16,
                        kind="ExternalOutput").ap()

    with tile.TileContext(nc) as tc, ExitStack() as ctx:
        cst = ctx.enter_context(tc.tile_pool(name="cst", bufs=1))
        sb = ctx.enter_context(tc.tile_pool(name="sb", bufs=2))
        ps = ctx.enter_context(tc.tile_pool(name="ps", bufs=8, space="PSUM"))

        # ---------------- prologue: constants & weights ----------------
        identf = cst.tile([128, 128], F32, tag="identf")
        make_identity(nc, identf[:])
        identh = cst.tile([128, 128], F16, tag="identh")
        nc.vector.tensor_copy(identh[:], identf[:])

        wenc = cst.tile([128, JC, D], F16, tag="wenc")
        nc.sync.dma_start(wenc[:], WENC)

        state = {}

        def prefetch_x(b):
            if b not in state:
                xt = sb.tile([128, JC, BLK], F16, tag="xt", bufs=2,
                             name="xt")
                nc.sync.dma_start(xt[:], XT[:, b])
                state[b] = dict(xt=xt, hr=[None] * JC, rt=[None] * MC)

        def encoder_half(b, j2):
            """Half the encoder (jc pair 2*j2, 2*j2+1); spread across the
            previous block so the ACT exp cadence never hiccups."""
            prefetch_x(b)
            st = state[b]
            xt = st["xt"]
            hp = ps.tile([128, 2, BLK], F32, tag="zp", bufs=4, name="hp")
            for half in range(2):
                jc = 2 * j2 + half
                for dc in range(JC):
                    nc.tensor.matmul(hp[:, half, :],
                                     lhsT=wenc[:, dc, ts(jc, 128)],
                                     rhs=xt[:, dc, :], start=(dc == 0),
                                     stop=(dc == JC - 1))
            hrt = sb.tile([128, 2, BLK], F16, tag=f"hr{j2}", bufs=2,
                          name="hrt")
            nc.scalar.activation(hrt[:], hp[:], AF.Copy, bias=0.0,
                                 scale=1.0)
            st["hr"][2 * j2] = hrt[:, 0, :]
            st["hr"][2 * j2 + 1] = hrt[:, 1, :]

        def load_gt(b, mc):
            gtp = sb.tile([128, KC, 2, BLK], F16, tag="gt", bufs=3,
                          name="gt")
            nc.sync.dma_start(gtp[:], GT[mc, :, b])
            return gtp

        def pair_scores(b, mc, gts=None):
            """Issue quad scores + gumbel injects for all kc of pair mc.
            Returns closures for the lagged recon steps + tail."""
            st = state[b]
            m0, m1 = 2 * mc, 2 * mc + 1
            gtp = gts if gts is not None else load_gt(b, mc)
            hr = st["hr"][mc]
            up2 = ps.tile([128, 2, BLK], F32, tag="zp", bufs=4, name="up")
            up0 = up2[:, 0, :]
            up1 = up2[:, 1, :]
            cfs = [None] * KC

            def step(kc):
                last = kc >= KC - 2
                zp = ps.tile([128, 2, BLK], F32, tag="zp", bufs=4, name="zp")
                # 2x2 quad: m0 on PE rows 0-63, m1 on rows 64-127; k-halves
                # on column groups. All four streams run concurrently.
                nc.tensor.matmul(zp[0:64, 0, :],
                                 lhsT=ct2[0:64, mc, kc, 0:64],
                                 rhs=hr[0:64, :], start=True, stop=last,
                                 tile_position=(0, 0), skip_group_check=True)
                nc.tensor.matmul(zp[64:128, 0, :],
                                 lhsT=ct2[0:64, mc, kc, 64:128],
                                 rhs=hr[0:64, :], start=True, stop=last,
                                 tile_position=(0, 64), skip_group_check=True)
                nc.tensor.matmul(zp[0:64, 1, :],
                                 lhsT=ct2[64:128, mc, kc, 0:64],
                                 rhs=hr[64:128, :], start=True, stop=last,
                                 tile_position=(64, 0), skip_group_check=True)
                nc.tensor.matmul(zp[64:128, 1, :],
                                 lhsT=ct2[64:128, mc, kc, 64:128],
                                 rhs=hr[64:128, :], start=True, stop=last,
                                 tile_position=(64, 64),
                                 skip_group_check=True)
                cf = sb.tile([128, 2, BLK], BF16, tag="cf", bufs=8, name="cf")
                if not last:
                    # gumbel(+bias) via identity matmuls, split into 64x64
                    # tiles at (0,0)/(64,64): the two tiles of each m run
                    # concurrently on disjoint PE quadrants.  (Tiles at the
                    # crossed positions (64,0)/(0,64) hang the HW when
                    # repeated -- empirically bisected.)
                    for half in range(2):
                        nc.tensor.matmul(zp[0:64, half, :],
                                         lhsT=identh[0:64, 0:64],
                                         rhs=gtp[0:64, kc, half, :],
                                         start=False, stop=True,
                                         tile_position=(0, 0),
                                         skip_group_check=True)
                        nc.tensor.matmul(zp[64:128, half, :],
                                         lhsT=identh[64:128, 64:128],
                                         rhs=gtp[64:128, kc, half, :],
                                         start=False, stop=True,
                                         tile_position=(64, 64),
                                         skip_group_check=True)
                    nc.scalar.activation(cf[:], zp[:], AF.Exp, bias=0.0,
                                         scale=1.0)
                else:
                    # last kc: gumbel via one wide DVE add (PE/DVE balance)
                    ez = sb.tile([128, 2, BLK], F32, tag="ez", bufs=3,
                                 name="ez")
                    nc.vector.tensor_add(ez[:], zp[:], gtp[:, kc, :, :])
                    nc.scalar.activation(cf[:], ez[:], AF.Exp, bias=0.0,
                                         scale=1.0)
                cfs[kc] = cf

            def recon(kc):
                last = kc >= KC - 2
                nc.tensor.matmul(up0[:], lhsT=cones[:, m0, kc, :],
                                 rhs=cfs[kc][:, 0, :], start=(kc == 0),
                                 stop=last)
                nc.tensor.matmul(up1[:], lhsT=cones[:, m1, kc, :],
                                 rhs=cfs[kc][:, 1, :], start=(kc == 0),
                                 stop=last)

            def tail():
                # 1/s straight off PSUM partition 0 (cones layout is
                # [ones|0*63|C]; the custom-DVE recip mis-lowers PSUM reads
                # at nonzero base partitions), then pool broadcast and a
                # fused drain+scale+cast multiply.
                rc0 = sb.tile([1, BLK], F32, tag="rc0", bufs=2, name="rc")
                rc1 = sb.tile([1, BLK], F32, tag="rc1", bufs=2, name="rc")
                nc.vector.reciprocal_approx_fast(rc0[:], up0[0:1, :])
                nc.vector.reciprocal_approx_fast(rc1[:], up1[0:1, :])
                bp0 = sb.tile([64, BLK], F32, tag="bp0", bufs=2, name="bp")
                nc.gpsimd.partition_broadcast(bp0[:], rc0[:], channels=64)
                bp1 = sb.tile([64, BLK], F32, tag="bp1", bufs=2, name="bp")
                nc.gpsimd.partition_broadcast(bp1[:], rc1[:], channels=64)
                rt = sb.tile([128, BLK], F16, tag=f"rt{mc}", bufs=2,
                             name="rt")
                with nc.allow_low_precision(reason="recon scale in fp16"):
                    nc.vector.tensor_mul(rt[0:64, :], up0[64:128, :], bp0[:])
                    nc.vector.tensor_mul(rt[64:128, :], up1[64:128, :],
                                         bp1[:])
                st["rt"][mc] = rt

            return step, recon, tail

        def decoder_half(b, j2):
            """Half the decoder (jc pair 2*j2, 2*j2+1), fp16 out."""
            st = state[b]
            if j2 == 0:
                st["yo"] = sb.tile([128, JC, BLK], F16, tag="yo", bufs=2,
                                   name="yo")
            yo = st["yo"]
            yp = ps.tile([128, 2, BLK], F32, tag="zp", bufs=4, name="yp")
            for half in range(2):
                jc = 2 * j2 + half
                for mcc in range(JC):
                    nc.tensor.matmul(yp[:, half, :],
                                     lhsT=wdec[:, mcc, ts(jc, 128)],
                                     rhs=st["rt"][mcc][:],
                                     start=(mcc == 0),
                                     stop=(mcc == JC - 1))
            with nc.allow_low_precision(reason="fp16 output"):
                nc.vector.tensor_copy(yo[:, 2 * j2:2 * j2 + 2, :], yp[:])
            nc.sync.dma_start(YT[:, b, 2 * j2:2 * j2 + 2], yo[:, 2 * j2:2 * j2 + 2, :])
            if j2 == 1:
                state.pop(b)

        # ---------------- schedule ----------------
        # Flat pipeline over (b, mc, kc).  Pair recons+tail drain as a
        # batch during kc==1 of the next pair (exp(mc,3) has landed by
        # then; the tail's DVE ops beat the next ez into the queue).
        # Encoder and decoder halves run on a dedicated PSUM tile and are
        # spread through the block so the exp cadence never hiccups.
        units = [(b, mc) for b in range(nblk) for mc in range(MC)]
        unit_objs = {}

        def get_unit(u):
            if u not in unit_objs:
                unit_objs[u] = pair_scores(*u)
            return unit_objs[u]

        def drain_unit(u):
            step, recon, tail = unit_objs.pop(u)
            for kc in range(KC):
                recon(kc)
            tail()

        encoder_half(0, 0)
        ct2 = cst.tile([128, MC, KC, 128], F16, tag="ct2")
        nc.sync.dma_start(ct2[:], CT2)
        get_unit((0, 0))         # prefetch pair-0 gumbel early
        encoder_half(0, 1)
        get_unit((0, 1))
        cones = cst.tile([128, M, KC, 128], BF16, tag="cones")
        nc.sync.dma_start(cones[:], CONES)
        wdec = cst.tile([128, JC, D], F16, tag="wdec")
        nc.sync.dma_start(wdec[:], WDEC)
        for ui, (b, mc) in enumerate(units):
            step, recon, tail = get_unit((b, mc))
            for kc in range(KC):
                if kc == 1 and ui > 0:
                    drain_unit(units[ui - 1])
                step(kc)
                if kc == 3:
                    if mc == 0 and b > 0:
                        decoder_half(b - 1, 0)
                    elif mc == 1 and b > 0:
                        decoder_half(b - 1, 1)
                    elif mc == 2 and b + 1 < nblk:
                        encoder_half(b + 1, 0)
                    elif mc == 3 and b + 1 < nblk:
                        encoder_half(b + 1, 1)
        drain_unit(units[-1])
        decoder_half(nblk - 1, 0)
        decoder_half(nblk - 1, 1)

    nc.compile()
    return nc


def _get_nc(nblk: int):
    key = ("nc", nblk)
    if key not in _CACHE:
        _CACHE[key] = build(nblk)
    return _CACHE[key]


def make_in_maps(inputs: dict, nblk: int):
    nloc = nblk * BLK
    f16 = ml_dtypes.float16 if hasattr(ml_dtypes, 'float16') else np.float16
    x = np.ascontiguousarray(inputs["x"], dtype=np.float32)
    g = np.ascontiguousarray(inputs["gumbel_noise"], dtype=np.float32)
    cb = np.ascontiguousarray(inputs["codebook"], dtype=np.float32)
    b_enc = np.ascontiguousarray(inputs["b_enc"], dtype=np.float32)
    b_dec = np.ascontiguousarray(inputs["b_dec"], dtype=np.float32)
    W_enc = np.ascontiguousarray(inputs["W_enc"], dtype=np.float32)
    W_dec = np.ascontiguousarray(inputs["W_dec"], dtype=np.float32)
    invT = np.exp(-np.asarray(inputs["log_temperatures"],
                              dtype=np.float64)).astype(np.float32)  # [M]

    # CT2[p, mc, kc, k]: p<64 -> m=2mc d=p; p>=64 -> m=2mc+1 d=p-64
    cbT = cb.transpose(0, 2, 1)                       # [M, DSUB, K]
    sc = (2.0 * invT)[:, None, None] * cbT            # [M, DSUB, K]
    sc = sc.reshape(MC, 2, DSUB, KC, 128)             # [mc, i, d, kc, k]
    ct2 = np.ascontiguousarray(
        sc.transpose(1, 2, 0, 3, 4).reshape(128, MC, KC, 128)
    ).astype(np.float16)

    # CONES[p, m, kc, 0] = 1; [p, m, kc, 64:128] = C[m, kc*128+p, :]
    # (ones column at out-partition 0, where the custom-DVE reciprocal can
    # read psum -- it mis-lowers psum reads at other base partitions; C at
    # out-partitions 64:128, the only legal 64-partition DVE base > 0)
    cns = np.zeros((128, M, KC, 128), dtype=np.float32)
    cns[:, :, :, 0] = 1.0
    cns[:, :, :, 64:128] = cb.reshape(M, KC, 128, DSUB).transpose(2, 0, 1, 3)
    cones = cns.astype(ml_dtypes.bfloat16)

    # gumbel with folded bias: g' = g - invT_m*|c_{m,k}|^2
    #                                  + 2*invT_m*(C[m,k] . b_enc_m)
    # (the b_enc fold lets the encoder drain be a pure cast: the score
    # z = 2invT C.(h0+b) differs from 2invT C.h0 by a per-(m,k) constant)
    cn2 = np.sum(cb * cb, axis=-1)                    # [M, K]
    cdotb = np.einsum('mkd,md->mk', cb, b_enc.reshape(M, DSUB))
    gbias = (invT[:, None] * (cn2 - 2.0 * cdotb))[None]   # [1, M, K]

    shared = dict(
        w_enc=np.ascontiguousarray(
            W_enc.reshape(JC, 128, D).transpose(1, 0, 2)).astype(np.float16),
        ct2=ct2,
        cones=cones,
        w_dec=np.ascontiguousarray(
            W_dec.reshape(JC, 128, D).transpose(1, 0, 2)).astype(np.float16),
    )
    in_maps = []
    for c in range(NCORES):
        lo = c * NLOC
        xc = x[lo:lo + nloc]                       # [nloc, D]
        xt = np.ascontiguousarray(
            xc.reshape(nblk, BLK, JC, 128).transpose(3, 0, 2, 1)
        ).astype(np.float16)
        gc = g[lo:lo + nloc] - gbias               # [nloc, M, K]
        # GT[mc, p, b, kc, m01, j] = g'[b*512+j, 2mc+m01, kc*128+p]
        gt = np.ascontiguousarray(
            gc.reshape(nblk, BLK, MC, 2, KC, 128).transpose(2, 5, 0, 4, 3, 1)
        ).astype(np.float16)
        in_maps.append(dict(shared, xt=xt, gumbel=gt))
    return in_maps


def run(inputs: dict, nblk: int = NLOC // BLK, trace: bool = False):
    from concourse.bass_utils import run_bass_kernel_spmd
    nc = _get_nc(nblk)
    b_dec_h = np.ascontiguousarray(inputs["b_dec"],
                                   dtype=np.float32)[None, :]
    in_maps = make_in_maps(inputs, nblk)
    res = run_bass_kernel_spmd(nc, in_maps, list(range(NCORES)), trace=trace)
    nloc = nblk * BLK
    out = np.empty((NCORES * nloc, D), dtype=np.float32)
    for c in range(NCORES):
        # YT[p, b, jc, j] -> y[b*512+j, jc*128+p]
        yt = np.asarray(res.results[c]["yt"], dtype=np.float32)
        out[c * nloc:(c + 1) * nloc] = np.ascontiguousarray(
            yt.transpose(1, 3, 2, 0).reshape(nloc, D)) + b_dec_h
    return out, res


def kernel(**inputs) -> np.ndarray:
    out, _ = run(inputs)
    return out


# revision 34
# speedup vs baseline: 1.1619x; 1.1619x over previous
"""DPQ (gumbel-softmax product-quantizer autoencoder) forward pass on 8
Trainium2 NeuronCores, data-parallel over the batch dimension.

Math (per row n, subspace m of 8, codebook of K=512 64-dim codes):
  h = x @ W_enc + b_enc                     [N, M*DSUB]
  score = (-|h|^2 + 2 h.c - |c|^2) / T_m    squared-distance scores
  codes = softmax(score + gumbel)           gumbel-softmax, TAU=1
  y = (codes @ C).flatten() @ W_dec + b_dec

v2 implementation notes (HW-profiled against the v1 kernel):
  * Everything runs transposed (n on the free dim); host pre-transposes all
    operands and untransposes the output, so no PE transposes anywhere.
  * The -|h|^2 term is constant over k so softmax cancels it; never computed.
  * The -invT*|c|^2 score bias is folded into the gumbel tensor ON THE HOST
    (free), so ACT exp needs no per-(m,kc) bias and can process both
    subspaces of a pair in one [128,1024] op spanning a 2-bank PSUM tile.
  * All matmul operands are 16-bit (fp16 for x/W/h/scores, bf16 where the
    fp32 exponent range is needed: exp outputs and the codebook in the recon
    matmul).  16-bit weights get a separate LDWEIGHTS that the PE's reorder
    window pulls ahead of in-flight matmuls, so weight loads cost ~0
    (f32r/f32 weights self-load serially, +60ns per matmul on HW).
  * Score matmuls run as 2x2 quad-tiled 64x64 PE tiles: m0 uses PE rows
    0-63, m1 rows 64-127, k-halves split over PE column groups.  All four
    64-contract streams run CONCURRENTLY (profiled ~234ns per quad vs
    ~430ns for the v1 row-paired version).
  * Gumbel noise (with the folded bias) rides identity matmuls into the
    PSUM accumulation for kc<3 and a DVE add for kc==3 (engine balance).
  * Softmax denominator s rides the recon matmul as a ones column (row 64
    of the 65-row up tiles).  1/s is computed by reciprocal_approx_fast
    (~600ns vs 3.4us for exact DVE reciprocal; 18 good bits), reading the
    PSUM row directly; the recon scaling multiply then reads the PSUM up
    tile and a 0-stride partition-broadcast AP of 1/s, fusing the psum
    drain, normalization and fp16 downcast into one DVE op per subspace.
  * Output is written fp16 (the v1 bf16 output cast dominated its error).
    End-to-end rel_absmax vs the fp32 reference: ~5e-3 (validated in numpy
    simulation and on HW), inside the 2e-2 gate.
"""

import sys
sys.path.insert(0, '/opt/trn_rl_repo')

import numpy as np
import ml_dtypes

N, D, M, K, DSUB = 32768, 512, 8, 512, 64
NCORES = 8
NLOC = N // NCORES          # rows per core
BLK = 512                   # rows per block
JC = D // 128               # 4 column chunks of 128
KC = K // 128               # 4 code chunks of 128
MC = M // 2                 # 4 subspace pairs

_CACHE = {}


def build(nblk: int):
    import concourse.bacc as bacc_mod
    import concourse.tile as tile
    import concourse.mybir as mybir
    from concourse.bass import ts
    from concourse.masks import make_identity
    from contextlib import ExitStack

    F32 = mybir.dt.float32
    F16 = mybir.dt.float16
    BF16 = mybir.dt.bfloat16
    AF = mybir.ActivationFunctionType

    nc = bacc_mod.Bacc(trn_type="TRN2", target_bir_lowering=False, debug=False)

    # Host-prepared layouts (see make_in_maps):
    #   XT[p, b, dc, j]      = fp16(x[b*512+j, dc*128+p])
    #   WENC[p, dc, j]       = fp16(W_enc[dc*128+p, j])
    #   AUX[p, 0:4]=b_enc, AUX[p, 4:8]=b_dec (per 128-chunk)
    #   CT2[p, mc, kc, k]    = fp16(2*invT_m*C[m, kc*128+k, d]), m/d from p
    #   CONES[p, m, kc, 0:64]= bf16(C[m, kc*128+p, :]), [...,64] = 1
    #   GT[m, p, b, kc, j]   = fp16(g[b*512+j, m, kc*128+p] - invT_m|c|^2)
    #   WDEC[p, mc, j]       = fp16(W_dec[mc*128+p, j])
    #   YT[p, b, jc, j]      = y[b*512+j, jc*128+p]            (output)
    XT = nc.dram_tensor("xt", [128, nblk, JC, BLK], F16,
                        kind="ExternalInput").ap()
    WENC = nc.dram_tensor("w_enc", [128, JC, D], F16,
                          kind="ExternalInput").ap()
    CT2 = nc.dram_tensor("ct2", [128, MC, KC, 128], F16,
                         kind="ExternalInput").ap()
    CONES = nc.dram_tensor("cones", [128, M, KC, 128], BF16,
                           kind="ExternalInput").ap()
    GT = nc.dram_tensor("gumbel", [MC, 128, nblk, KC, 2, BLK], F16,
                        kind="ExternalInput").ap()
    WDEC = nc.dram_tensor("w_dec", [128, JC, D], F16,
                          kind="ExternalInput").ap()
    YT = nc.dram_tensor("yt", [128, nblk, JC, BLK], F16,
                        kind="ExternalOutput").ap()

    with tile.TileContext(nc) as tc, ExitStack() as ctx:
        cst = ctx.enter_context(tc.tile_pool(name="cst", bufs=1))
        sb = ctx.enter_context(tc.tile_pool(name="sb", bufs=2))
        ps = ctx.enter_context(tc.tile_pool(name="ps", bufs=8, space="PSUM"))

        # ---------------- prologue: constants & weights ----------------
        identf = cst.tile([128, 128], F32, tag="identf")
        make_identity(nc, identf[:])
        identh = cst.tile([128, 128], F16, tag="identh")
        nc.vector.tensor_copy(identh[:], identf[:])

        wenc = cst.tile([128, JC, D], F16, tag="wenc")
        nc.sync.dma_start(wenc[:], WENC)

        state = {}

        def prefetch_x(b):
            if b not in state:
                xt = sb.tile([128, JC, BLK], F16, tag="xt", bufs=2,
                             name="xt")
                nc.sync.dma_start(xt[:], XT[:, b])
                state[b] = dict(xt=xt, hr=[None] * JC, rt=[None] * MC)

        def encoder_half(b, j2):
            """Half the encoder (jc pair 2*j2, 2*j2+1); spread across the
            previous block so the ACT exp cadence never hiccups."""
            prefetch_x(b)
            st = state[b]
            xt = st["xt"]
            hp = ps.tile([128, 2, BLK], F32, tag="zp", bufs=4, name="hp")
            for half in range(2):
                jc = 2 * j2 + half
                for dc in range(JC):
                    nc.tensor.matmul(hp[:, half, :],
                                     lhsT=wenc[:, dc, ts(jc, 128)],
                                     rhs=xt[:, dc, :], start=(dc == 0),
                                     stop=(dc == JC - 1))
            hrt = sb.tile([128, 2, BLK], F16, tag=f"hr{j2}", bufs=2,
                          name="hrt")
            nc.scalar.activation(hrt[:], hp[:], AF.Copy, bias=0.0,
                                 scale=1.0)
            st["hr"][2 * j2] = hrt[:, 0, :]
            st["hr"][2 * j2 + 1] = hrt[:, 1, :]

        def load_gt(b, mc):
            gtp = sb.tile([128, KC, 2, BLK], F16, tag="gt", bufs=3,
                          name="gt")
            nc.sync.dma_start(gtp[:], GT[mc, :, b])
            return gtp

        def pair_scores(b, mc, gts=None):
            """Issue quad scores + gumbel injects for all kc of pair mc.
            Returns closures for the lagged recon steps + tail."""
            st = state[b]
            m0, m1 = 2 * mc, 2 * mc + 1
            gtp = gts if gts is not None else load_gt(b, mc)
            hr = st["hr"][mc]
            up2 = ps.tile([128, 2, BLK], F32, tag="zp", bufs=4, name="up")
            up0 = up2[:, 0, :]
            up1 = up2[:, 1, :]
            cfs = [None] * KC

            def step(kc):
                last = kc == KC - 1
                zp = ps.tile([128, 2, BLK], F32, tag="zp", bufs=4, name="zp")
                # 2x2 quad: m0 on PE rows 0-63, m1 on rows 64-127; k-halves
                # on column groups. All four streams run concurrently.
                nc.tensor.matmul(zp[0:64, 0, :],
                                 lhsT=ct2[0:64, mc, kc, 0:64],
                                 rhs=hr[0:64, :], start=True, stop=last,
                                 tile_position=(0, 0), skip_group_check=True)
                nc.tensor.matmul(zp[64:128, 0, :],
                                 lhsT=ct2[0:64, mc, kc, 64:128],
                                 rhs=hr[0:64, :], start=True, stop=last,
                                 tile_position=(0, 64), skip_group_check=True)
                nc.tensor.matmul(zp[0:64, 1, :],
                                 lhsT=ct2[64:128, mc, kc, 0:64],
                                 rhs=hr[64:128, :], start=True, stop=last,
                                 tile_position=(64, 0), skip_group_check=True)
                nc.tensor.matmul(zp[64:128, 1, :],
                                 lhsT=ct2[64:128, mc, kc, 64:128],
                                 rhs=hr[64:128, :], start=True, stop=last,
                                 tile_position=(64, 64),
                                 skip_group_check=True)
                cf = sb.tile([128, 2, BLK], BF16, tag="cf", bufs=8, name="cf")
                if not last:
                    # gumbel(+bias) via identity matmuls, split into 64x64
                    # tiles at (0,0)/(64,64): the two tiles of each m run
                    # concurrently on disjoint PE quadrants.  (Tiles at the
                    # crossed positions (64,0)/(0,64) hang the HW when
                    # repeated -- empirically bisected.)
                    for half in range(2):
                        nc.tensor.matmul(zp[0:64, half, :],
                                         lhsT=identh[0:64, 0:64],
                                         rhs=gtp[0:64, kc, half, :],
                                         start=False, stop=True,
                                         tile_position=(0, 0),
                                         skip_group_check=True)
                        nc.tensor.matmul(zp[64:128, half, :],
                                         lhsT=identh[64:128, 64:128],
                                         rhs=gtp[64:128, kc, half, :],
                                         start=False, stop=True,
                                         tile_position=(64, 64),
                                         skip_group_check=True)
                    nc.scalar.activation(cf[:], zp[:], AF.Exp, bias=0.0,
                                         scale=1.0)
                else:
                    # last kc: gumbel via one wide DVE add (PE/DVE balance)
                    ez = sb.tile([128, 2, BLK], F32, tag="ez", bufs=3,
                                 name="ez")
                    nc.vector.tensor_add(ez[:], zp[:], gtp[:, kc, :, :])
                    nc.scalar.activation(cf[:], ez[:], AF.Exp, bias=0.0,
                                         scale=1.0)
                cfs[kc] = cf

            def recon(kc):
                last = kc == KC - 1
                nc.tensor.matmul(up0[:], lhsT=cones[:, m0, kc, :],
                                 rhs=cfs[kc][:, 0, :], start=(kc == 0),
                                 stop=last)
                nc.tensor.matmul(up1[:], lhsT=cones[:, m1, kc, :],
                                 rhs=cfs[kc][:, 1, :], start=(kc == 0),
                                 stop=last)

            def tail():
                # 1/s straight off PSUM partition 0 (cones layout is
                # [ones|0*63|C]; the custom-DVE recip mis-lowers PSUM reads
                # at nonzero base partitions), then pool broadcast and a
                # fused drain+scale+cast multiply.
                rc0 = sb.tile([1, BLK], F32, tag="rc0", bufs=2, name="rc")
                rc1 = sb.tile([1, BLK], F32, tag="rc1", bufs=2, name="rc")
                nc.vector.reciprocal_approx_fast(rc0[:], up0[0:1, :])
                nc.vector.reciprocal_approx_fast(rc1[:], up1[0:1, :])
                bp0 = sb.tile([64, BLK], F32, tag="bp0", bufs=2, name="bp")
                nc.gpsimd.partition_broadcast(bp0[:], rc0[:], channels=64)
                bp1 = sb.tile([64, BLK], F32, tag="bp1", bufs=2, name="bp")
                nc.gpsimd.partition_broadcast(bp1[:], rc1[:], channels=64)
                rt = sb.tile([128, BLK], F16, tag=f"rt{mc}", bufs=2,
                             name="rt")
                with nc.allow_low_precision(reason="recon scale in fp16"):
                    nc.vector.tensor_mul(rt[0:64, :], up0[64:128, :], bp0[:])
                    nc.vector.tensor_mul(rt[64:128, :], up1[64:128, :],
                                         bp1[:])
                st["rt"][mc] = rt

            return step, recon, tail

        def decoder_half(b, j2):
            """Half the decoder (jc pair 2*j2, 2*j2+1), fp16 out."""
            st = state[b]
            if j2 == 0:
                st["yo"] = sb.tile([128, JC, BLK], F16, tag="yo", bufs=2,
                                   name="yo")
            yo = st["yo"]
            yp = ps.tile([128, 2, BLK], F32, tag="zp", bufs=4, name="yp")
            for half in range(2):
                jc = 2 * j2 + half
                for mcc in range(JC):
                    nc.tensor.matmul(yp[:, half, :],
                                     lhsT=wdec[:, mcc, ts(jc, 128)],
                                     rhs=st["rt"][mcc][:],
                                     start=(mcc == 0),
                                     stop=(mcc == JC - 1))
            with nc.allow_low_precision(reason="fp16 output"):
                nc.vector.tensor_copy(yo[:, 2 * j2:2 * j2 + 2, :], yp[:])
            nc.sync.dma_start(YT[:, b, 2 * j2:2 * j2 + 2], yo[:, 2 * j2:2 * j2 + 2, :])
            if j2 == 1:
                state.pop(b)

        # ---------------- schedule ----------------
        # Flat pipeline over (b, mc, kc).  Pair recons+tail drain as a
        # batch during kc==1 of the next pair (exp(mc,3) has landed by
        # then; the tail's DVE ops beat the next ez into the queue).
        # Encoder and decoder halves run on a dedicated PSUM tile and are
        # spread through the block so the exp cadence never hiccups.
        units = [(b, mc) for b in range(nblk) for mc in range(MC)]
        unit_objs = {}

        def get_unit(u):
            if u not in unit_objs:
                unit_objs[u] = pair_scores(*u)
            return unit_objs[u]

        def drain_unit(u):
            step, recon, tail = unit_objs.pop(u)
            for kc in range(KC):
                recon(kc)
            tail()

        encoder_half(0, 0)
        ct2 = cst.tile([128, MC, KC, 128], F16, tag="ct2")
        nc.sync.dma_start(ct2[:], CT2)
        get_unit((0, 0))         # prefetch pair-0 gumbel early
        encoder_half(0, 1)
        get_unit((0, 1))
        cones = cst.tile([128, M, KC, 128], BF16, tag="cones")
        nc.sync.dma_start(cones[:], CONES)
        wdec = cst.tile([128, JC, D], F16, tag="wdec")
        nc.sync.dma_start(wdec[:], WDEC)
        for ui, (b, mc) in enumerate(units):
            step, recon, tail = get_unit((b, mc))
            for kc in range(KC):
                if kc == 1 and ui > 0:
                    drain_unit(units[ui - 1])
                step(kc)
                if kc == 3:
                    if mc == 0 and b > 0:
                        decoder_half(b - 1, 0)
                    elif mc == 1 and b > 0:
                        decoder_half(b - 1, 1)
                    elif mc == 2 and b + 1 < nblk:
                        encoder_half(b + 1, 0)
                    elif mc == 3 and b + 1 < nblk:
                        encoder_half(b + 1, 1)
        drain_unit(units[-1])
        decoder_half(nblk - 1, 0)
        decoder_half(nblk - 1, 1)

    nc.compile()
    return nc


def _get_nc(nblk: int):
    key = ("nc", nblk)
    if key not in _CACHE:
        _CACHE[key] = build(nblk)
    return _CACHE[key]


def make_in_maps(inputs: dict, nblk: int):
    nloc = nblk * BLK
    f16 = ml_dtypes.float16 if hasattr(ml_dtypes, 'float16') else np.float16
    x = np.ascontiguousarray(inputs["x"], dtype=np.float32)
    g = np.ascontiguousarray(inputs["gumbel_noise"], dtype=np.float32)
    cb = np.ascontiguousarray(inputs["codebook"], dtype=np.float32)
    b_enc = np.ascontiguousarray(inputs["b_enc"], dtype=np.float32)
    b_dec = np.ascontiguousarray(inputs["b_dec"], dtype=np.float32)
    W_enc = np.ascontiguousarray(inputs["W_enc"], dtype=np.float32)
    W_dec = np.ascontiguousarray(inputs["W_dec"], dtype=np.float32)
    invT = np.exp(-np.asarray(inputs["log_temperatures"],
                              dtype=np.float64)).astype(np.float32)  # [M]

    # CT2[p, mc, kc, k]: p<64 -> m=2mc d=p; p>=64 -> m=2mc+1 d=p-64
    cbT = cb.transpose(0, 2, 1)                       # [M, DSUB, K]
    sc = (2.0 * invT)[:, None, None] * cbT            # [M, DSUB, K]
    sc = sc.reshape(MC, 2, DSUB, KC, 128)             # [mc, i, d, kc, k]
    ct2 = np.ascontiguousarray(
        sc.transpose(1, 2, 0, 3, 4).reshape(128, MC, KC, 128)
    ).astype(np.float16)

    # CONES[p, m, kc, 0] = 1; [p, m, kc, 64:128] = C[m, kc*128+p, :]
    # (ones column at out-partition 0, where the custom-DVE reciprocal can
    # read psum -- it mis-lowers psum reads at other base partitions; C at
    # out-partitions 64:128, the only legal 64-partition DVE base > 0)
    cns = np.zeros((128, M, KC, 128), dtype=np.float32)
    cns[:, :, :, 0] = 1.0
    cns[:, :, :, 64:128] = cb.reshape(M, KC, 128, DSUB).transpose(2, 0, 1, 3)
    cones = cns.astype(ml_dtypes.bfloat16)

    # gumbel with folded bias: g' = g - invT_m*|c_{m,k}|^2
    #                                  + 2*invT_m*(C[m,k] . b_enc_m)
    # (the b_enc fold lets the encoder drain be a pure cast: the score
    # z = 2invT C.(h0+b) differs from 2invT C.h0 by a per-(m,k) constant)
    cn2 = np.sum(cb * cb, axis=-1)                    # [M, K]
    cdotb = np.einsum('mkd,md->mk', cb, b_enc.reshape(M, DSUB))
    gbias = (invT[:, None] * (cn2 - 2.0 * cdotb))[None]   # [1, M, K]

    shared = dict(
        w_enc=np.ascontiguousarray(
            W_enc.reshape(JC, 128, D).transpose(1, 0, 2)).astype(np.float16),
        ct2=ct2,
        cones=cones,
        w_dec=np.ascontiguousarray(
            W_dec.reshape(JC, 128, D).transpose(1, 0, 2)).astype(np.float16),
    )
    in_maps = []
    for c in range(NCORES):
        lo = c * NLOC
        xc = x[lo:lo + nloc]                       # [nloc, D]
        xt = np.ascontiguousarray(
            xc.reshape(nblk, BLK, JC, 128).transpose(3, 0, 2, 1)
        ).astype(np.float16)
        gc = g[lo:lo + nloc] - gbias               # [nloc, M, K]
        # GT[mc, p, b, kc, m01, j] = g'[b*512+j, 2mc+m01, kc*128+p]
        gt = np.ascontiguousarray(
            gc.reshape(nblk, BLK, MC, 2, KC, 128).transpose(2, 5, 0, 4, 3, 1)
        ).astype(np.float16)
        in_maps.append(dict(shared, xt=xt, gumbel=gt))
    return in_maps


def run(inputs: dict, nblk: int = NLOC // BLK, trace: bool = False):
    from concourse.bass_utils import run_bass_kernel_spmd
    nc = _get_nc(nblk)
    b_dec_h = np.ascontiguousarray(inputs["b_dec"],
                                   dtype=np.float32)[None, :]
    in_maps = make_in_maps(inputs, nblk)
    res = run_bass_kernel_spmd(nc, in_maps, list(range(NCORES)), trace=trace)
    nloc = nblk * BLK
    out = np.empty((NCORES * nloc, D), dtype=np.float32)
    for c in range(NCORES):
        # YT[p, b, jc, j] -> y[b*512+j, jc*128+p]
        yt = np.asarray(res.results[c]["yt"], dtype=np.float32)
        out[c * nloc:(c + 1) * nloc] = np.ascontiguousarray(
            yt.transpose(1, 3, 2, 0).reshape(nloc, D)) + b_dec_h
    return out, res


def kernel(**inputs) -> np.ndarray:
    out, _ = run(inputs)
    return out


# revision 36
# speedup vs baseline: 1.2082x; 1.0398x over previous
"""DPQ (gumbel-softmax product-quantizer autoencoder) forward pass on 8
Trainium2 NeuronCores, data-parallel over the batch dimension.

Math (per row n, subspace m of 8, codebook of K=512 64-dim codes):
  h = x @ W_enc + b_enc                     [N, M*DSUB]
  score = (-|h|^2 + 2 h.c - |c|^2) / T_m    squared-distance scores
  codes = softmax(score + gumbel)           gumbel-softmax, TAU=1
  y = (codes @ C).flatten() @ W_dec + b_dec

v2 implementation notes (HW-profiled against the v1 kernel):
  * Everything runs transposed (n on the free dim); host pre-transposes all
    operands and untransposes the output, so no PE transposes anywhere.
  * The -|h|^2 term is constant over k so softmax cancels it; never computed.
  * The -invT*|c|^2 score bias is folded into the gumbel tensor ON THE HOST
    (free), so ACT exp needs no per-(m,kc) bias and can process both
    subspaces of a pair in one [128,1024] op spanning a 2-bank PSUM tile.
  * All matmul operands are 16-bit (fp16 for x/W/h/scores, bf16 where the
    fp32 exponent range is needed: exp outputs and the codebook in the recon
    matmul).  16-bit weights get a separate LDWEIGHTS that the PE's reorder
    window pulls ahead of in-flight matmuls, so weight loads cost ~0
    (f32r/f32 weights self-load serially, +60ns per matmul on HW).
  * Score matmuls run as 2x2 quad-tiled 64x64 PE tiles: m0 uses PE rows
    0-63, m1 rows 64-127, k-halves split over PE column groups.  All four
    64-contract streams run CONCURRENTLY (profiled ~234ns per quad vs
    ~430ns for the v1 row-paired version).
  * Gumbel noise (with the folded bias) rides identity matmuls into the
    PSUM accumulation for kc<3 and a DVE add for kc==3 (engine balance).
  * Softmax denominator s rides the recon matmul as a ones column (row 64
    of the 65-row up tiles).  1/s is computed by reciprocal_approx_fast
    (~600ns vs 3.4us for exact DVE reciprocal; 18 good bits), reading the
    PSUM row directly; the recon scaling multiply then reads the PSUM up
    tile and a 0-stride partition-broadcast AP of 1/s, fusing the psum
    drain, normalization and fp16 downcast into one DVE op per subspace.
  * Output is written fp16 (the v1 bf16 output cast dominated its error).
    End-to-end rel_absmax vs the fp32 reference: ~5e-3 (validated in numpy
    simulation and on HW), inside the 2e-2 gate.
"""

import sys
sys.path.insert(0, '/opt/trn_rl_repo')

import numpy as np
import ml_dtypes

N, D, M, K, DSUB = 32768, 512, 8, 512, 64
NCORES = 8
NLOC = N // NCORES          # rows per core
BLK = 512                   # rows per block
JC = D // 128               # 4 column chunks of 128
KC = K // 128               # 4 code chunks of 128
MC = M // 2                 # 4 subspace pairs

_CACHE = {}


def build(nblk: int):
    import concourse.bacc as bacc_mod
    import concourse.tile as tile
    import concourse.mybir as mybir
    from concourse.bass import ts
    from concourse.masks import make_identity
    from contextlib import ExitStack

    F32 = mybir.dt.float32
    F16 = mybir.dt.float16
    BF16 = mybir.dt.bfloat16
    AF = mybir.ActivationFunctionType

    nc = bacc_mod.Bacc(trn_type="TRN2", target_bir_lowering=False, debug=False)

    # Host-prepared layouts (see make_in_maps):
    #   XT[p, b, dc, j]      = fp16(x[b*512+j, dc*128+p])
    #   WENC[p, dc, j]       = fp16(W_enc[dc*128+p, j])
    #   AUX[p, 0:4]=b_enc, AUX[p, 4:8]=b_dec (per 128-chunk)
    #   CT2[p, mc, kc, k]    = fp16(2*invT_m*C[m, kc*128+k, d]), m/d from p
    #   CONES[p, m, kc, 0:64]= bf16(C[m, kc*128+p, :]), [...,64] = 1
    #   GT[m, p, b, kc, j]   = fp16(g[b*512+j, m, kc*128+p] - invT_m|c|^2)
    #   WDEC[p, mc, j]       = fp16(W_dec[mc*128+p, j])
    #   YT[p, b, jc, j]      = y[b*512+j, jc*128+p]            (output)
    XT = nc.dram_tensor("xt", [128, nblk, JC, BLK], F16,
                        kind="ExternalInput").ap()
    WENC = nc.dram_tensor("w_enc", [128, JC, D], F16,
                          kind="ExternalInput").ap()
    CT2 = nc.dram_tensor("ct2", [128, MC, KC, 128], F16,
                         kind="ExternalInput").ap()
    CONES = nc.dram_tensor("cones", [128, M, KC, 128], BF16,
                           kind="ExternalInput").ap()
    GT = nc.dram_tensor("gumbel", [MC, 128, nblk, KC, 2, BLK], F16,
                        kind="ExternalInput").ap()
    WDEC = nc.dram_tensor("w_dec", [128, JC, D], F16,
                          kind="ExternalInput").ap()
    YT = nc.dram_tensor("yt", [128, nblk, JC, BLK], F16,
                        kind="ExternalOutput").ap()

    with tile.TileContext(nc) as tc, ExitStack() as ctx:
        cst = ctx.enter_context(tc.tile_pool(name="cst", bufs=1))
        sb = ctx.enter_context(tc.tile_pool(name="sb", bufs=2))
        ps = ctx.enter_context(tc.tile_pool(name="ps", bufs=8, space="PSUM"))

        # ---------------- prologue: constants & weights ----------------
        identf = cst.tile([128, 128], F32, tag="identf")
        make_identity(nc, identf[:])
        identh = cst.tile([128, 128], F16, tag="identh")
        nc.vector.tensor_copy(identh[:], identf[:])

        wenc = cst.tile([128, JC, D], F16, tag="wenc")
        nc.sync.dma_start(wenc[:], WENC)

        state = {}

        def prefetch_x(b):
            if b not in state:
                xt = sb.tile([128, JC, BLK], F16, tag="xt", bufs=2,
                             name="xt")
                nc.sync.dma_start(xt[:], XT[:, b])
                state[b] = dict(xt=xt, hr=[None] * JC, rt=[None] * MC)

        def encoder_half(b, j2):
            """Half the encoder (jc pair 2*j2, 2*j2+1); spread across the
            previous block so the ACT exp cadence never hiccups."""
            prefetch_x(b)
            st = state[b]
            xt = st["xt"]
            hp = ps.tile([128, 2, BLK], F32, tag="zp", bufs=4, name="hp")
            for half in range(2):
                jc = 2 * j2 + half
                for dc in range(JC):
                    nc.tensor.matmul(hp[:, half, :],
                                     lhsT=wenc[:, dc, ts(jc, 128)],
                                     rhs=xt[:, dc, :], start=(dc == 0),
                                     stop=(dc == JC - 1))
            hrt = sb.tile([128, 2, BLK], F16, tag=f"hr{j2}", bufs=2,
                          name="hrt")
            nc.scalar.activation(hrt[:], hp[:], AF.Copy, bias=0.0,
                                 scale=1.0)
            st["hr"][2 * j2] = hrt[:, 0, :]
            st["hr"][2 * j2 + 1] = hrt[:, 1, :]

        def load_gt(b, mc):
            gtp = sb.tile([128, KC, 2, BLK], F16, tag="gt", bufs=3,
                          name="gt")
            nc.sync.dma_start(gtp[:], GT[mc, :, b])
            return gtp

        def pair_scores(b, mc, gts=None):
            """Issue quad scores + gumbel injects for all kc of pair mc.
            Returns closures for the lagged recon steps + tail."""
            st = state[b]
            m0, m1 = 2 * mc, 2 * mc + 1
            gtp = gts if gts is not None else load_gt(b, mc)
            hr = st["hr"][mc]
            up2 = ps.tile([128, 2, BLK], F32, tag="zp", bufs=4, name="up")
            up0 = up2[:, 0, :]
            up1 = up2[:, 1, :]
            cfs = [None] * KC

            def step(kc):
                zp = ps.tile([128, 2, BLK], F32, tag="zp", bufs=4, name="zp")
                # 2x2 quad: m0 on PE rows 0-63, m1 on rows 64-127; k-halves
                # on column groups. All four streams run concurrently.
                nc.tensor.matmul(zp[0:64, 0, :],
                                 lhsT=ct2[0:64, mc, kc, 0:64],
                                 rhs=hr[0:64, :], start=True, stop=False,
                                 tile_position=(0, 0), skip_group_check=True)
                nc.tensor.matmul(zp[64:128, 0, :],
                                 lhsT=ct2[0:64, mc, kc, 64:128],
                                 rhs=hr[0:64, :], start=True, stop=False,
                                 tile_position=(0, 64), skip_group_check=True)
                nc.tensor.matmul(zp[0:64, 1, :],
                                 lhsT=ct2[64:128, mc, kc, 0:64],
                                 rhs=hr[64:128, :], start=True, stop=False,
                                 tile_position=(64, 0), skip_group_check=True)
                nc.tensor.matmul(zp[64:128, 1, :],
                                 lhsT=ct2[64:128, mc, kc, 64:128],
                                 rhs=hr[64:128, :], start=True, stop=False,
                                 tile_position=(64, 64),
                                 skip_group_check=True)
                cf = sb.tile([128, 2, BLK], BF16, tag="cf", bufs=8, name="cf")
                # gumbel(+bias) via identity matmuls, split into 64x64
                # tiles at (0,0)/(64,64): the two tiles of each m run
                # concurrently on disjoint PE quadrants.  (Tiles at the
                # crossed positions (64,0)/(0,64) hang the HW when
                # repeated -- empirically bisected.)
                for half in range(2):
                    nc.tensor.matmul(zp[0:64, half, :],
                                     lhsT=identh[0:64, 0:64],
                                     rhs=gtp[0:64, kc, half, :],
                                     start=False, stop=True,
                                     tile_position=(0, 0),
                                     skip_group_check=True)
                    nc.tensor.matmul(zp[64:128, half, :],
                                     lhsT=identh[64:128, 64:128],
                                     rhs=gtp[64:128, kc, half, :],
                                     start=False, stop=True,
                                     tile_position=(64, 64),
                                     skip_group_check=True)
                nc.scalar.activation(cf[:], zp[:], AF.Exp, bias=0.0,
                                     scale=1.0)
                cfs[kc] = cf

            def recon(kc):
                last = kc == KC - 1
                nc.tensor.matmul(up0[:], lhsT=cones[:, m0, kc, :],
                                 rhs=cfs[kc][:, 0, :], start=(kc == 0),
                                 stop=last)
                nc.tensor.matmul(up1[:], lhsT=cones[:, m1, kc, :],
                                 rhs=cfs[kc][:, 1, :], start=(kc == 0),
                                 stop=last)

            def tail():
                # 1/s straight off PSUM partition 0 (cones layout is
                # [ones|0*63|C]; the custom-DVE recip mis-lowers PSUM reads
                # at nonzero base partitions), then pool broadcast and a
                # fused drain+scale+cast multiply.
                rc0 = sb.tile([1, BLK], F32, tag="rc0", bufs=2, name="rc")
                rc1 = sb.tile([1, BLK], F32, tag="rc1", bufs=2, name="rc")
                nc.vector.reciprocal_approx_fast(rc0[:], up0[0:1, :])
                nc.vector.reciprocal_approx_fast(rc1[:], up1[0:1, :])
                bp0 = sb.tile([64, BLK], F32, tag="bp0", bufs=2, name="bp")
                nc.gpsimd.partition_broadcast(bp0[:], rc0[:], channels=64)
                bp1 = sb.tile([64, BLK], F32, tag="bp1", bufs=2, name="bp")
                nc.gpsimd.partition_broadcast(bp1[:], rc1[:], channels=64)
                rt = sb.tile([128, BLK], F16, tag=f"rt{mc}", bufs=2,
                             name="rt")
                with nc.allow_low_precision(reason="recon scale in fp16"):
                    nc.vector.tensor_mul(rt[0:64, :], up0[64:128, :], bp0[:])
                    nc.vector.tensor_mul(rt[64:128, :], up1[64:128, :],
                                         bp1[:])
                st["rt"][mc] = rt

            return step, recon, tail

        def decoder_half(b, j2):
            """Half the decoder (jc pair 2*j2, 2*j2+1), fp16 out."""
            st = state[b]
            if j2 == 0:
                st["yo"] = sb.tile([128, JC, BLK], F16, tag="yo", bufs=2,
                                   name="yo")
            yo = st["yo"]
            yp = ps.tile([128, 2, BLK], F32, tag="zp", bufs=4, name="yp")
            for half in range(2):
                jc = 2 * j2 + half
                for mcc in range(JC):
                    nc.tensor.matmul(yp[:, half, :],
                                     lhsT=wdec[:, mcc, ts(jc, 128)],
                                     rhs=st["rt"][mcc][:],
                                     start=(mcc == 0),
                                     stop=(mcc == JC - 1))
            with nc.allow_low_precision(reason="fp16 output"):
                nc.vector.tensor_copy(yo[:, 2 * j2:2 * j2 + 2, :], yp[:])
            nc.sync.dma_start(YT[:, b, 2 * j2:2 * j2 + 2], yo[:, 2 * j2:2 * j2 + 2, :])
            if j2 == 1:
                state.pop(b)

        # ---------------- schedule ----------------
        # Flat pipeline over (b, mc, kc).  Pair recons+tail drain as a
        # batch during kc==1 of the next pair (exp(mc,3) has landed by
        # then; the tail's DVE ops beat the next ez into the queue).
        # Encoder and decoder halves run on a dedicated PSUM tile and are
        # spread through the block so the exp cadence never hiccups.
        units = [(b, mc) for b in range(nblk) for mc in range(MC)]
        unit_objs = {}

        def get_unit(u):
            if u not in unit_objs:
                unit_objs[u] = pair_scores(*u)
            return unit_objs[u]

        def drain_unit(u):
            step, recon, tail = unit_objs.pop(u)
            for kc in range(KC):
                recon(kc)
            tail()

        encoder_half(0, 0)
        ct2 = cst.tile([128, MC, KC, 128], F16, tag="ct2")
        nc.sync.dma_start(ct2[:], CT2)
        get_unit((0, 0))         # prefetch pair-0 gumbel early
        encoder_half(0, 1)
        get_unit((0, 1))
        cones = cst.tile([128, M, KC, 128], BF16, tag="cones")
        nc.sync.dma_start(cones[:], CONES)
        wdec = cst.tile([128, JC, D], F16, tag="wdec")
        nc.sync.dma_start(wdec[:], WDEC)
        for ui, (b, mc) in enumerate(units):
            step, recon, tail = get_unit((b, mc))
            for kc in range(KC):
                if kc == 1 and ui > 0:
                    drain_unit(units[ui - 1])
                step(kc)
                if kc == 3:
                    if mc == 0 and b > 0:
                        decoder_half(b - 1, 0)
                    elif mc == 1 and b > 0:
                        decoder_half(b - 1, 1)
                    elif mc == 2 and b + 1 < nblk:
                        encoder_half(b + 1, 0)
                    elif mc == 3 and b + 1 < nblk:
                        encoder_half(b + 1, 1)
        drain_unit(units[-1])
        decoder_half(nblk - 1, 0)
        decoder_half(nblk - 1, 1)

    nc.compile()
    return nc


def _get_nc(nblk: int):
    key = ("nc", nblk)
    if key not in _CACHE:
        _CACHE[key] = build(nblk)
    return _CACHE[key]


def make_in_maps(inputs: dict, nblk: int):
    nloc = nblk * BLK
    f16 = ml_dtypes.float16 if hasattr(ml_dtypes, 'float16') else np.float16
    x = np.ascontiguousarray(inputs["x"], dtype=np.float32)
    g = np.ascontiguousarray(inputs["gumbel_noise"], dtype=np.float32)
    cb = np.ascontiguousarray(inputs["codebook"], dtype=np.float32)
    b_enc = np.ascontiguousarray(inputs["b_enc"], dtype=np.float32)
    b_dec = np.ascontiguousarray(inputs["b_dec"], dtype=np.float32)
    W_enc = np.ascontiguousarray(inputs["W_enc"], dtype=np.float32)
    W_dec = np.ascontiguousarray(inputs["W_dec"], dtype=np.float32)
    invT = np.exp(-np.asarray(inputs["log_temperatures"],
                              dtype=np.float64)).astype(np.float32)  # [M]

    # CT2[p, mc, kc, k]: p<64 -> m=2mc d=p; p>=64 -> m=2mc+1 d=p-64
    cbT = cb.transpose(0, 2, 1)                       # [M, DSUB, K]
    sc = (2.0 * invT)[:, None, None] * cbT            # [M, DSUB, K]
    sc = sc.reshape(MC, 2, DSUB, KC, 128)             # [mc, i, d, kc, k]
    ct2 = np.ascontiguousarray(
        sc.transpose(1, 2, 0, 3, 4).reshape(128, MC, KC, 128)
    ).astype(np.float16)

    # CONES[p, m, kc, 0] = 1; [p, m, kc, 64:128] = C[m, kc*128+p, :]
    # (ones column at out-partition 0, where the custom-DVE reciprocal can
    # read psum -- it mis-lowers psum reads at other base partitions; C at
    # out-partitions 64:128, the only legal 64-partition DVE base > 0)
    cns = np.zeros((128, M, KC, 128), dtype=np.float32)
    cns[:, :, :, 0] = 1.0
    cns[:, :, :, 64:128] = cb.reshape(M, KC, 128, DSUB).transpose(2, 0, 1, 3)
    cones = cns.astype(ml_dtypes.bfloat16)

    # gumbel with folded bias: g' = g - invT_m*|c_{m,k}|^2
    #                                  + 2*invT_m*(C[m,k] . b_enc_m)
    # (the b_enc fold lets the encoder drain be a pure cast: the score
    # z = 2invT C.(h0+b) differs from 2invT C.h0 by a per-(m,k) constant)
    cn2 = np.sum(cb * cb, axis=-1)                    # [M, K]
    cdotb = np.einsum('mkd,md->mk', cb, b_enc.reshape(M, DSUB))
    gbias = (invT[:, None] * (cn2 - 2.0 * cdotb))[None]   # [1, M, K]

    shared = dict(
        w_enc=np.ascontiguousarray(
            W_enc.reshape(JC, 128, D).transpose(1, 0, 2)).astype(np.float16),
        ct2=ct2,
        cones=cones,
        w_dec=np.ascontiguousarray(
            W_dec.reshape(JC, 128, D).transpose(1, 0, 2)).astype(np.float16),
    )
    in_maps = []
    for c in range(NCORES):
        lo = c * NLOC
        xc = x[lo:lo + nloc]                       # [nloc, D]
        xt = np.ascontiguousarray(
            xc.reshape(nblk, BLK, JC, 128).transpose(3, 0, 2, 1)
        ).astype(np.float16)
        gc = g[lo:lo + nloc] - gbias               # [nloc, M, K]
        # GT[mc, p, b, kc, m01, j] = g'[b*512+j, 2mc+m01, kc*128+p]
        gt = np.ascontiguousarray(
            gc.reshape(nblk, BLK, MC, 2, KC, 128).transpose(2, 5, 0, 4, 3, 1)
        ).astype(np.float16)
        in_maps.append(dict(shared, xt=xt, gumbel=gt))
    return in_maps


def run(inputs: dict, nblk: int = NLOC // BLK, trace: bool = False):
    from concourse.bass_utils import run_bass_kernel_spmd
    nc = _get_nc(nblk)
    b_dec_h = np.ascontiguousarray(inputs["b_dec"],
                                   dtype=np.float32)[None, :]
    in_maps = make_in_maps(inputs, nblk)
    res = run_bass_kernel_spmd(nc, in_maps, list(range(NCORES)), trace=trace)
    nloc = nblk * BLK
    out = np.empty((NCORES * nloc, D), dtype=np.float32)
    for c in range(NCORES):
        # YT[p, b, jc, j] -> y[b*512+j, jc*128+p]
        yt = np.asarray(res.results[c]["yt"], dtype=np.float32)
        out[c * nloc:(c + 1) * nloc] = np.ascontiguousarray(
            yt.transpose(1, 3, 2, 0).reshape(nloc, D)) + b_dec_h
    return out, res


def kernel(**inputs) -> np.ndarray:
    out, _ = run(inputs)
    return out
